# revision 23
# baseline (speedup 1.0000x reference)
# Emu3 VQVAE vector-quantizer kernel for 8x TRN2 NeuronCores (Bass/Tile).
#
# Problem: hidden_state (8,1,256,32,32) f32, codebook (16384,256) f32
#   -> nearest-codebook-entry indices (8,1,32,32) int32
#   distances = |x|^2 + |e|^2 - 2 x.e ; argmin over K with first-index ties.
#
# Numerics: |e|^2 ~ 3e-7 while |x|^2 ~ 256, so in fp32 (xsq + esq) == xsq
# bitwise. The reference distances are d = fl(xsq - fl(2*mm)); ~4% of rows
# have exact fp32 ties at the min, so we reproduce the quantized d values
# and first-index tie-breaking.
#
# Sharding: data-parallel over the 8 batch entries (1024 tokens each);
# codebook replicated.
#
# Per core, per (section, token-tile): PE matmul (fp32r: 1 cycle/row, 2
# C-half passes) accumulates 2*mm into a 2048-wide PSUM tile. Default
# variant "fused_act":
#   ACT : d = fl(xsq - psum)        (the reference's quantized distance;
#                                    ACT reads PSUM without stalling PE)
#   DVE : ONE custom op VQ_NEGKEYS_MAX1 (registered at import into the ant
#         custom-DVE table): body = (base - d)*(g_p/u_p) - Idx, accum=max
#         -> accum_out = -min_k((d-base)*g_p/u_p + k), i.e. a packed
#         (quantized-distance, index) argmin key per 2048-wide section.
# u_p = per-partition-row ulp scale (max of the row's 8 tokens' base
# ulps; tokens are pre-sorted by binade so the ratio is 1, 2, rarely 4);
# g_p in {8192, 4096} chosen per row so the key stays an exact fp32
# integer < 2^24. The key min is the lexicographic (d, k)-min =
# first-index argmin. ("fused" variant: single 2-input custom op straight
# from PSUM - fewer ACT ops but DVE-PSUM reads serialize against PE.)
#
# Decode (batched, float/bitwise-exact): key_int = -minik = n'*2048+kmod;
# cross-section winner via key2 = n'*8 + s (exact < 2^17), then
# mask-select kmod of the winning section. (Plain int32 tensor adds are
# executed in fp32 by the DVE - values above 2^24 lose low bits - so the
# decode only uses bitwise ops and exact-in-fp32 arithmetic.)

import numpy as np

B, T, C, H, W = 8, 1, 256, 32, 32
K = 16384
NCORES = 8
NTOK = H * W          # tokens per core
NTILES = NTOK // 128  # token tiles per core
CHUNK = 512
SECW = 2048           # argmin section width (11 index bits)
NSECT = K // SECW     # 8

_CACHE = {}
_VQ_OPS = {}


def _register_op(name, spec):
    from concourse import dve_ops
    from concourse.dve_spec import lower, _has_src1
    from concourse.dve_uop import DveOpSpec

    if name in _VQ_OPS:
        return _VQ_OPS[name]
    if name not in dve_ops._SUB_OPCODE_FOR_NAME:
        row = max(dve_ops._SUB_OPCODE_FOR_NAME.values()) + 1
        assert row < 0x20
        dve_ops._SUB_OPCODE_FOR_NAME[name] = row
    shas = {}
    for ver in ("v3", "v4"):
        try:
            s = DveOpSpec(
                name=name,
                opcode=dve_ops.get_dve_sub_opcode(name),
                uops=lower(spec, ver=ver),
                rd1_en=_has_src1(spec),
            )
            shas[ver] = s.sha(ver)
        except Exception:
            pass
    assert shas, f"{name} failed to lower for all DVE vers"
    op = dve_ops.DveOp(name, spec, subdim=False, uops_sha=shas)
    if all(o.name != name for o in dve_ops.OPS):
        dve_ops.OPS.append(op)
    dve_ops.CUSTOM_DVE_SPECS[name] = spec
    _VQ_OPS[name] = op
    return op


def _register_vq_op():
    """Fused negated-key max-reduce straight from PSUM:
    body = (base - (xsq - psum)) - iotaP, accum = max."""
    from concourse.dve_spec import C0, C1, Spec, Src0, Src1, maxx

    def _ref(in0, in1, s0, s1, imm2):
        d = (s0 - in0.astype(np.float32)).astype(np.float32)
        body = ((s1 - d).astype(np.float32) - in1).astype(np.float32)
        acc = body.reshape(body.shape[0], -1).max(axis=-1, keepdims=True)
        return body, acc

    return _register_op(
        "VQ_NEGKEY_MAX", Spec(body=(C1 - (C0 - Src0)) - Src1, accum=maxx,
                              reference=_ref)
    )


def _register_vq_op_1in():
    """Fused scaled negated-key max-reduce from an SBUF d-slab:
    body = (base - d)*(g/u_p) - Idx, accum = max (single input port)."""
    from concourse.dve_spec import C0, C1, Idx, Spec, Src0, maxx

    def _ref(in0, in1, s0, s1, imm2):
        idx = np.arange(in0.shape[-1], dtype=np.float32)
        body = (((s0 - in0.astype(np.float32)).astype(np.float32) * s1)
                .astype(np.float32) - idx).astype(np.float32)
        acc = body.reshape(body.shape[0], -1).max(axis=-1, keepdims=True)
        return body, acc

    return _register_op(
        "VQ_NEGKEYS_MAX1", Spec(body=(C0 - Src0) * C1 - Idx, accum=maxx,
                                reference=_ref)
    )


SPLIT = 8  # sections per 64 whose quantize runs on DVE instead of ACT


def _build_bass(mode="float32r", repeats=1, variant="fused", ablate="none"):
    from contextlib import ExitStack

    import concourse.bass as bass  # noqa: F401
    import concourse.mybir as mybir
    import concourse.tile as tile
    from concourse import bacc

    f32 = mybir.dt.float32
    bf16 = mybir.dt.bfloat16
    i32 = mybir.dt.int32
    is_bf16x3 = mode == "bf16x3"
    mm_dt = bf16 if is_bf16x3 else getattr(mybir.dt, mode)
    AF = mybir.ActivationFunctionType
    ALU = mybir.AluOpType
    vq_op = _register_vq_op()
    vq_op1 = _register_vq_op_1in()

    nc = bacc.Bacc(
        "TRN2",
        target_bir_lowering=False,
        debug=False,
        enable_asserts=False,
        num_devices=NCORES,
    )

    NS = 2 if is_bf16x3 else 1
    xT_d = nc.dram_tensor("xT", (NS, 2, 128, NTOK), mm_dt, kind="ExternalInput").ap()
    cb_d = nc.dram_tensor("cbT2", (NS, 2, 128, K), mm_dt, kind="ExternalInput").ap()
    xsq_d = nc.dram_tensor("xsqp", (128, NTILES), f32, kind="ExternalInput").ap()
    base_d = nc.dram_tensor("base", (128, NTILES), f32, kind="ExternalInput").ap()
    iot_d = nc.dram_tensor("iotaP", (128, SECW), f32, kind="ExternalInput").ap()
    scn_d = nc.dram_tensor("scalN", (128, 1), f32, kind="ExternalInput").ap()
    scp_d = nc.dram_tensor("scalP", (128, 1), f32, kind="ExternalInput").ap()
    sps_d = nc.dram_tensor("spatS", (128, NTILES, NSECT), f32, kind="ExternalInput").ap()
    spw_d = nc.dram_tensor("spatW", (128, NTILES, NSECT), f32, kind="ExternalInput").ap()
    out_d = nc.dram_tensor("idx", (128, NTILES), i32, kind="ExternalOutput").ap()

    if is_bf16x3:
        TERMS = [(0, 0, 0), (0, 0, 1), (0, 1, 0), (0, 1, 1), (1, 0, 0), (1, 0, 1)]
    else:
        TERMS = [(0, 0, 0), (0, 0, 1)]

    with tile.TileContext(nc) as tc:
        with ExitStack() as ctx:
            cbp = ctx.enter_context(tc.tile_pool(name="cb", bufs=1))
            xp = ctx.enter_context(tc.tile_pool(name="x", bufs=1))
            pp = ctx.enter_context(tc.tile_pool(name="psum", bufs=2, space="PSUM"))
            smp = ctx.enter_context(tc.tile_pool(name="small", bufs=2))

            xts = {}
            for hl in range(NS):
                for cs in range(2):
                    xt = xp.tile([128, NTOK], mm_dt, tag=f"x{hl}_{cs}")
                    nc.sync.dma_start(xt[:], xT_d[hl][cs])
                    xts[hl, cs] = xt
            xsq = xp.tile([128, NTILES], f32, tag="xsq")
            nc.sync.dma_start(xsq[:], xsq_d[:])
            base = xp.tile([128, NTILES], f32, tag="base")
            nc.sync.dma_start(base[:], base_d[:])
            iotaP = xp.tile([128, SECW], f32, tag="iotaP")
            nc.sync.dma_start(iotaP[:], iot_d[:])
            scalN = xp.tile([128, 1], f32, tag="scalN")
            nc.sync.dma_start(scalN[:], scn_d[:])
            scalP = xp.tile([128, 1], f32, tag="scalP")
            nc.sync.dma_start(scalP[:], scp_d[:])
            spatS = xp.tile([128, NTILES, NSECT], f32, tag="spatS")
            nc.sync.dma_start(spatS[:], sps_d[:])
            spatW = xp.tile([128, NTILES, NSECT], f32, tag="spatW")
            nc.sync.dma_start(spatW[:], spw_d[:])

            # codebook: one DMA per (section, hi/lo, C-half) so compute on
            # early sections overlaps DMA of later ones (sec-outer loop).
            cbs = {}
            for s in range(NSECT):
                for hl in range(NS):
                    for cs in range(2):
                        cbt = cbp.tile([128, SECW], mm_dt, tag=f"cb{hl}_{cs}_{s}")
                        nc.sync.dma_start(
                            cbt[:], cb_d[hl][cs][:, s * SECW : (s + 1) * SECW]
                        )
                        cbs[hl, cs, s] = cbt

            # int consts for the batched decode
            c2047 = xp.tile([128, NTILES, NSECT], i32, tag="c2047")
            nc.vector.memset(c2047[:], SECW - 1)
            cm2048 = xp.tile([128, NTILES, NSECT], i32, tag="cm2048")
            nc.vector.memset(cm2048[:], -SECW)
            c7 = xp.tile([128, NTILES], i32, tag="c7")
            nc.vector.memset(c7[:], NSECT - 1)

            trash = xp.tile([128, SECW], f32, tag="trash")

            dp = ctx.enter_context(tc.tile_pool(name="dslab", bufs=3))
            zslab = None
            if ablate == "dveonly":
                zslab = xp.tile([128, SECW], f32, tag="zslab")
                nc.vector.memset(zslab[:], 1.0)
            for rep in range(repeats):
                minik = smp.tile([128, NTILES, NSECT], f32, tag="minik")
                if ablate == "peonly":
                    nc.vector.memset(minik[:], 1.0)
                for sec in range(NSECT):
                    for t in range(NTILES):
                        if ablate == "dveonly" and variant == "fused_act":
                            # pure 1-input custom-op rate from SBUF
                            nc.vector._custom_dve(
                                vq_op1,
                                out=trash[:],
                                in0=zslab[:],
                                s0=base[:, t : t + 1],
                                s1=scalP[:],
                                accum_out=minik[:, t, sec : sec + 1],
                            )
                            continue
                        ps = pp.tile(
                            [128, SECW], f32, tag="ps", name=f"ps_{rep}_{sec}_{t}"
                        )
                        if ablate == "dveonly":
                            nc.scalar.activation(ps[:], zslab[:], AF.Identity)
                        else:
                            for ti, (xhl, ehl, cs) in enumerate(TERMS):
                                for ci in range(SECW // CHUNK):
                                    nc.tensor.matmul(
                                        ps[:, ci * CHUNK : (ci + 1) * CHUNK],
                                        xts[xhl, cs][:, t * 128 : (t + 1) * 128],
                                        cbs[ehl, cs, sec][
                                            :, ci * CHUNK : (ci + 1) * CHUNK
                                        ],
                                        start=(ti == 0),
                                        stop=(ti == len(TERMS) - 1),
                                    )
                        if ablate == "peonly":
                            continue
                        if variant == "fused_act":
                            dsl = dp.tile([128, SECW], f32, tag="dsl")
                            unit = sec * NTILES + t
                            stride = (NSECT * NTILES) // SPLIT if SPLIT else 0
                            if SPLIT and unit % stride == stride - 1:
                                # quantize on DVE: d = (psum - xsq) * -1
                                nc.vector.tensor_scalar(
                                    dsl[:], ps[:], xsq[:, t : t + 1], -1.0,
                                    op0=ALU.subtract, op1=ALU.mult,
                                )
                            else:
                                nc.scalar.activation(
                                    dsl[:], ps[:], AF.Identity,
                                    bias=xsq[:, t : t + 1], scale=-1.0,
                                )
                            nc.vector._custom_dve(
                                vq_op1,
                                out=trash[:],
                                in0=dsl[:],
                                s0=base[:, t : t + 1],
                                s1=scalP[:],
                                accum_out=minik[:, t, sec : sec + 1],
                            )
                        elif variant == "fused":
                            # one DVE op: negkey = (base-(xsq-psum)) - iotaP,
                            # accum_out = max -> -min over packed (d,k) keys
                            nc.vector._custom_dve(
                                vq_op,
                                out=trash[:],
                                in0=ps[:],
                                in1=iotaP[:],
                                s0=xsq[:, t : t + 1],
                                s1=base[:, t : t + 1],
                                accum_out=minik[:, t, sec : sec + 1],
                            )
                        else:
                            # unfused fallback: ACT quantize, DVE negate-
                            # and-shift, DVE iota-subtract, DVE max-reduce
                            dsl = smp.tile([128, SECW], f32, tag="dsl")
                            nc.scalar.activation(
                                dsl[:], ps[:], AF.Identity,
                                bias=xsq[:, t : t + 1], scale=-1.0,
                            )
                            nc.vector.tensor_scalar(
                                dsl[:], dsl[:], base[:, t : t + 1], -1.0,
                                op0=ALU.subtract, op1=ALU.mult,
                            )
                            nc.vector.tensor_tensor(
                                dsl[:], dsl[:], iotaP[:], op=ALU.subtract
                            )
                            nc.vector.tensor_reduce(
                                minik[:, t, sec : sec + 1], dsl[:],
                                axis=mybir.AxisListType.X, op=ALU.max,
                            )

                # ---- batched decode (all ops fp32-exact or pure bitwise) ----
                # key_int = -minik * scalN = n'*2048 + kmod  (< 2^24)
                keyf = smp.tile([128, NTILES, NSECT], f32, tag="keyf")
                nc.vector.tensor_scalar(
                    keyf[:], minik[:], scalN[:], None, op0=ALU.mult
                )
                mi = smp.tile([128, NTILES, NSECT], i32, tag="mi")
                nc.vector.tensor_copy(mi[:], keyf[:])
                kmi = smp.tile([128, NTILES, NSECT], i32, tag="kmi")
                nc.vector.tensor_tensor(kmi[:], mi[:], c2047[:], op=ALU.bitwise_and)
                nwi = smp.tile([128, NTILES, NSECT], i32, tag="nwi")
                nc.vector.tensor_tensor(nwi[:], mi[:], cm2048[:], op=ALU.bitwise_and)
                kmf = smp.tile([128, NTILES, NSECT], f32, tag="kmf")
                nc.vector.tensor_copy(kmf[:], kmi[:])
                nwf = smp.tile([128, NTILES, NSECT], f32, tag="nwf")
                nc.vector.tensor_copy(nwf[:], nwi[:])
                # key2 = n'*8 + s  (exact, < 2^17)
                key2 = smp.tile([128, NTILES, NSECT], f32, tag="key2")
                nc.vector.tensor_scalar(
                    key2[:], nwf[:], 1.0 / 256.0, None, op0=ALU.mult
                )
                nc.vector.tensor_tensor(key2[:], key2[:], spatS[:], op=ALU.add)
                m2 = smp.tile([128, NTILES], f32, tag="m2")
                nc.vector.tensor_reduce(
                    m2[:], key2[:], axis=mybir.AxisListType.X, op=ALU.min
                )
                m2i = smp.tile([128, NTILES], i32, tag="m2i")
                nc.vector.tensor_copy(m2i[:], m2[:])
                ssi = smp.tile([128, NTILES], i32, tag="ssi")
                nc.vector.tensor_tensor(ssi[:], m2i[:], c7[:], op=ALU.bitwise_and)
                ssf = smp.tile([128, NTILES, 1], f32, tag="ssf")
                nc.vector.tensor_copy(ssf[:], ssi[:])
                mask = smp.tile([128, NTILES, NSECT], f32, tag="mask")
                nc.vector.tensor_tensor(
                    mask[:], spatS[:], ssf.broadcast_to((128, NTILES, NSECT)),
                    op=ALU.is_equal,
                )
                cand = smp.tile([128, NTILES, NSECT], f32, tag="cand")
                nc.vector.tensor_tensor(cand[:], kmf[:], spatW[:], op=ALU.add)
                nc.vector.tensor_tensor(cand[:], cand[:], mask[:], op=ALU.mult)
                idxf = smp.tile([128, NTILES], f32, tag="idxf")
                nc.vector.tensor_reduce(
                    idxf[:], cand[:], axis=mybir.AxisListType.X, op=ALU.add
                )
                idxi = smp.tile([128, NTILES], i32, tag="idxi")
                nc.vector.tensor_copy(idxi[:], idxf[:])
                nc.sync.dma_start(out_d[:], idxi[:])

    nc.compile()
    return nc


def get_nc(mode="float32r", repeats=1, variant="fused", ablate="none"):
    key = ("nc", mode, repeats, variant, ablate, SPLIT)
    if key not in _CACHE:
        _CACHE[key] = _build_bass(mode, repeats, variant, ablate)
    return _CACHE[key]


def prepare_inputs(hidden_state, codebook, mode="float32r", variant=None):
    if variant is None:
        variant = VARIANT
    """Host-side shard prep: returns in_maps (list of 8 dicts)."""
    import ml_dtypes

    hs = np.ascontiguousarray(np.asarray(hidden_state, dtype=np.float32))
    cb = np.ascontiguousarray(np.asarray(codebook, dtype=np.float32))
    xT = hs.reshape(B, C, NTOK)
    cb2 = (2.0 * cb.T).astype(np.float32)  # (C, K), exact doubling
    if mode == "bf16x3":
        cb2h = cb2.astype(ml_dtypes.bfloat16)
        cb2l = (cb2 - cb2h.astype(np.float32)).astype(ml_dtypes.bfloat16)
        cb_in = np.ascontiguousarray(np.stack([cb2h, cb2l]).reshape(2, 2, 128, K))
    else:
        cb_in = np.ascontiguousarray(cb2.reshape(1, 2, 128, K))

    s_row = np.arange(NSECT, dtype=np.float32).reshape(1, 1, NSECT)
    spatS = np.ascontiguousarray(
        np.broadcast_to(s_row, (128, NTILES, NSECT)).astype(np.float32)
    )
    spatW = np.ascontiguousarray((spatS * SECW).astype(np.float32))

    # |2*e_k| bound for the per-token distance-spread budget
    emax = float(np.max(np.linalg.norm(2.0 * cb.astype(np.float64), axis=1)))

    in_maps = []
    perms = []
    for b in range(B):
        xb32_orig = xT[b]
        xsq_o = np.sum(xb32_orig * xb32_orig, axis=0, dtype=np.float32)  # (NTOK,)

        # base_t <= min_k d (Cauchy-Schwarz bound, 2% margin)
        xsq64_o = xsq_o.astype(np.float64)
        bound_o = np.sqrt(xsq64_o) * emax * 1.02 + 1e-6
        base_o = (xsq64_o - bound_o).astype(np.float32)
        _, exp_o = np.frexp(base_o)  # (NTOK,) binade exponents

        # Sort tokens by binade so each partition row (8 tokens: (p, t=0..7))
        # is binade-homogeneous up to one octave. SBUF position t*128+p holds
        # sorted token p*NTILES+t; pos2orig maps position -> original token.
        order = np.argsort(exp_o, kind="stable")
        i = np.arange(NTOK)
        pos2orig = np.empty(NTOK, dtype=np.int64)
        pos2orig[(i % NTILES) * 128 + i // NTILES] = order[i]
        perms.append(pos2orig)

        xb32 = np.ascontiguousarray(xb32_orig[:, pos2orig])
        xsq = xsq_o[pos2orig]
        xsq64 = xsq64_o[pos2orig]
        bound = bound_o[pos2orig]
        base = base_o[pos2orig]
        exp = exp_o[pos2orig]
        ulp64 = np.ldexp(np.float64(1.0), exp - 24)

        if mode == "bf16x3":
            xh = xb32.astype(ml_dtypes.bfloat16)
            xl = (xb32 - xh.astype(np.float32)).astype(ml_dtypes.bfloat16)
            xin = np.ascontiguousarray(np.stack([xh, xl]).reshape(2, 2, 128, NTOK))
        else:
            xin = np.ascontiguousarray(xb32.reshape(1, 2, 128, NTOK))

        def pt(a):  # (NTOK,) -> (128, NTILES)
            return np.ascontiguousarray(a.reshape(NTILES, 128).T)

        # per-partition-row ulp scale u_p = max over the row's tokens
        exp_p = pt(exp.astype(np.float64)).max(axis=1)  # (128,)
        up64 = np.ldexp(np.float64(1.0), exp_p.astype(np.int64) - 24)
        ratio = up64[:, None] / pt(ulp64)  # (128, NTILES)
        assert np.isin(ratio, (1.0, 2.0, 4.0)).all(), (
            f"base binade spread > 2 octaves within a partition row: "
            f"ratios {np.unique(ratio)}"
        )
        assert (base > 0).all()
        spread = xsq64 + bound - base.astype(np.float64)  # (NTOK,) permuted
        spread_pt = pt(spread)  # (128, NTILES)
        # Per-row iota granularity g_p: key_int = (d-base)*g_p/u_p + k must
        # stay < 2^24. Prefer g=8192 (clean tie-bits even for ratio-4
        # tokens); fall back to 4096 where 8192 busts the budget.
        budget = 2.0**24 - SECW
        need = spread_pt.max(axis=1) / up64  # (128,) worst dq per row
        g_p = np.where(need * 8192.0 < budget, 8192.0, 4096.0)
        assert (need * g_p < budget).all(), (
            f"distance-spread exceeds key budget; max {(need * g_p).max():.0f}"
        )
        # rows where a ratio-4 token coexists with g=4096 lose exact
        # tie-break/kmod for that token - must be rare
        polluted = ((ratio >= 4.0) & (g_p[:, None] == 4096.0)).sum()
        assert polluted < 64, f"too many budget-conflicted tokens: {polluted}"

        iotaP = np.ascontiguousarray(
            (np.arange(SECW, dtype=np.float64)[None, :] * (up64 / g_p)[:, None])
            .astype(np.float32)
        )
        scalP = np.ascontiguousarray(((g_p / up64)[:, None]).astype(np.float32))
        if variant == "fused_act":
            scalN = np.full((128, 1), -1.0, dtype=np.float32)
        else:
            scalN = np.ascontiguousarray(
                (-(g_p / up64)[:, None]).astype(np.float32)
            )

        in_maps.append(
            {
                "xT": xin,
                "cbT2": cb_in,
                "xsqp": pt(xsq),
                "base": pt(base),
                "iotaP": iotaP,
                "scalN": scalN,
                "scalP": scalP,
                "spatS": spatS,
                "spatW": spatW,
            }
        )
    return in_maps, perms


MODE = "float32r"
VARIANT = "fused_act"


def kernel(hidden_state, codebook):
    from concourse.bass_utils import run_bass_kernel_spmd

    nc = get_nc(MODE, 1, VARIANT)
    in_maps, perms = prepare_inputs(hidden_state, codebook, MODE, VARIANT)
    res = run_bass_kernel_spmd(nc, in_maps, core_ids=list(range(NCORES)))
    # idx is (128, NTILES): token position = t*128 + p lives at [p, t];
    # un-permute back to original token order.
    out = np.empty((B, NTOK), dtype=np.int32)
    for b in range(B):
        out[b, perms[b]] = res.results[b]["idx"].T.reshape(NTOK)
    return out.reshape(B, T, H, W)
